# revision 5
# baseline (speedup 1.0000x reference)
"""Trainium2 Bass kernel for a 2-stage 13-organ Dice loss.

Math (all organ weights are 1.0, so the per-organ fold collapses to sums):
  for stage s, batch b:
    num[s,b] = 2 * sum_{c in 1..13} sum_v pred_s[b,c,v] * [target[b,v]==c]
    den[s,b] = sum_{c in 1..13} sum_v pred_s[b,c,v]^2 + count(target[b]!=0) + 13*EPS
  dice[b] = num[1,b]/den[1,b] + num[2,b]/den[2,b]
  loss    = mean_b(2 - dice[b])

Sharding: the 48-slice depth axis is split 6-per-core across 8 NeuronCores;
each core handles both batches, both stages, and organ channels 1..13
(channel 0 is background and never touches the device). Each core emits
per-partition partial sums (a few KB); the host does the final reduction
and dice division.

Per-core device program (Tile framework; no PE/PSUM needed):
  - DVE builds 13 one-hot masks per target slab with tensor_scalar(is_equal)
    (2x fp32 perf mode), plus a zero-count via a fused accum_out.
  - DVE tensor_tensor_reduce fuses (pred * mask) * 2 with the per-partition
    numerator sum in one 1x pass.
  - ACT activation(Square) computes squares with a fused per-partition
    accum_out for the denominator.
All reductions land in small f32 "slot" tiles that are DMA'd out.
"""

import numpy as np

import concourse.bacc as bacc
import concourse.bass as bass
import concourse.mybir as mybir
import concourse.tile as tile
from concourse.bass_utils import run_bass_kernel_spmd

N_CORES = 8
S = 2  # stages
B = 2  # batch
C = 13  # organ channels (pred channels 1..13; channel 0 skipped)
D = 48  # depth
D_SH = D // N_CORES  # 6 depth slices per core
HW = 256 * 256  # voxels per (b, d) slab
PJ = HW // 128  # 512 free elems per partition per slab
EPS = 1e-5

F32 = mybir.dt.float32
BF16 = mybir.dt.bfloat16


def build_program(d_sh: int = D_SH, pj: int = PJ) -> bass.Bass:
    """Build the per-core SPMD Bass program.

    Inputs  (per core): pred [S,B,C,d_sh,128*pj] f32, tgt [B,d_sh,128*pj] f32
    Outputs (per core): onum [128,32] f32  (slot idx = ((b*d_sh)+d)*S + s)
                        oden [128,32] f32  (same slot layout)
                        ocnt [128,16] f32  (slot idx = b*d_sh + d; holds
                                            per-partition counts of target==0)
    """
    hw = 128 * pj
    nc = bacc.Bacc(target_bir_lowering=False)
    pred = nc.dram_tensor("pred", [S, B, C, d_sh, hw], F32, kind="ExternalInput")
    tgt = nc.dram_tensor("tgt", [B, d_sh, hw], F32, kind="ExternalInput")
    onum = nc.dram_tensor("onum", [128, 32], F32, kind="ExternalOutput")
    oden = nc.dram_tensor("oden", [128, 32], F32, kind="ExternalOutput")
    ocnt = nc.dram_tensor("ocnt", [128, 16], F32, kind="ExternalOutput")

    with tile.TileContext(nc) as tc:
        with (
            tc.tile_pool(name="tpool", bufs=2) as tpool,
            tc.tile_pool(name="ppool", bufs=3) as ppool,
            tc.tile_pool(name="mpool", bufs=2) as mpool,
            tc.tile_pool(name="dpool", bufs=2) as dpool,
            tc.tile_pool(name="spool", bufs=1) as spool,
        ):
            num_slots = spool.tile([128, 32], F32, tag="num")
            den_slots = spool.tile([128, 32], F32, tag="den")
            cnt_slots = spool.tile([128, 16], F32, tag="cnt")
            # Unused slot columns are DMA'd out; zero them so outputs are
            # deterministic.
            nc.vector.memset(num_slots[:, :], 0.0)
            nc.vector.memset(den_slots[:, :], 0.0)
            nc.vector.memset(cnt_slots[:, :], 0.0)

            for b in range(B):
                tb = tpool.tile([128, d_sh, pj], F32, tag="tb")
                nc.sync.dma_start(
                    out=tb[:, :, :],
                    in_=tgt[b].rearrange("d (p j) -> p d j", p=128),
                )
                for d in range(d_sh):
                    masks = mpool.tile([128, C, pj], BF16, tag="masks")
                    for c in range(C):
                        nc.vector.tensor_scalar(
                            masks[:, c, :],
                            tb[:, d, :],
                            float(c + 1),
                            None,
                            mybir.AluOpType.is_equal,
                        )
                    zslot = b * d_sh + d
                    zdummy = dpool.tile([128, pj], BF16, tag="zd")
                    nc.vector.tensor_scalar(
                        zdummy[:, :],
                        tb[:, d, :],
                        0.0,
                        None,
                        mybir.AluOpType.is_equal,
                        mybir.AluOpType.add,
                        accum_out=cnt_slots[:, zslot : zslot + 1],
                    )
                    for s in range(S):
                        slot = (b * d_sh + d) * S + s
                        pt = ppool.tile([128, C, pj], F32, tag="pt")
                        nc.sync.dma_start(
                            out=pt[:, :, :],
                            in_=pred[s, b][:, d, :].rearrange("c (p j) -> p c j", p=128),
                        )
                        sdummy = dpool.tile([128, C, pj], BF16, tag="sd")
                        nc.scalar.activation(
                            sdummy[:, :, :],
                            pt[:, :, :],
                            mybir.ActivationFunctionType.Square,
                            accum_out=den_slots[:, slot : slot + 1],
                        )
                        mdummy = dpool.tile([128, C, pj], BF16, tag="md")
                        # out = (pt * 2.0) * mask; accum_out = per-partition sum
                        nc.vector.scalar_tensor_tensor(
                            out=mdummy[:, :, :],
                            in0=pt[:, :, :],
                            scalar=2.0,
                            in1=masks[:, :, :],
                            op0=mybir.AluOpType.mult,
                            op1=mybir.AluOpType.mult,
                            accum_out=num_slots[:, slot : slot + 1],
                        )

            nc.sync.dma_start(out=onum[:, :], in_=num_slots[:, :])
            nc.sync.dma_start(out=oden[:, :], in_=den_slots[:, :])
            nc.sync.dma_start(out=ocnt[:, :], in_=cnt_slots[:, :])
    nc.finalize()
    return nc


def shard_inputs(pred_stage1, pred_stage2, target, n_cores=N_CORES, d_sh=D_SH):
    """Slice off the background channel and split the depth axis per core."""
    in_maps = []
    for k in range(n_cores):
        d0, d1 = k * d_sh, (k + 1) * d_sh
        pshard = np.empty((S, B, C, d_sh, HW), np.float32)
        pshard[0] = np.asarray(pred_stage1[:, 1:, d0:d1]).reshape(B, C, d_sh, HW)
        pshard[1] = np.asarray(pred_stage2[:, 1:, d0:d1]).reshape(B, C, d_sh, HW)
        tshard = np.asarray(target[:, d0:d1]).reshape(B, d_sh, HW).astype(np.float32)
        in_maps.append({"pred": pshard, "tgt": tshard})
    return in_maps


def combine_results(results, d_sh=D_SH, pj=PJ):
    """Host-side final reduction of the per-core per-partition partials."""
    num = np.zeros((S, B), np.float64)
    den = np.zeros((S, B), np.float64)
    cnt = np.zeros((B,), np.float64)
    slab_voxels = 128 * pj
    for r in results:
        onum = r["onum"].astype(np.float64)
        oden = r["oden"].astype(np.float64)
        ocnt = r["ocnt"].astype(np.float64)
        for b in range(B):
            for d in range(d_sh):
                cnt[b] += slab_voxels - ocnt[:, b * d_sh + d].sum()
                for s in range(S):
                    slot = (b * d_sh + d) * S + s
                    num[s, b] += onum[:, slot].sum()
                    den[s, b] += oden[:, slot].sum()
    dice = np.zeros(B, np.float64)
    for b in range(B):
        for s in range(S):
            dice[b] += num[s, b] / (den[s, b] + cnt[b] + C * EPS)
    loss = np.mean(2.0 - dice)
    return np.array(loss, dtype=np.float32)


def kernel(pred_stage1, pred_stage2, target):
    in_maps = shard_inputs(pred_stage1, pred_stage2, target)
    nc = build_program()
    res = run_bass_kernel_spmd(nc, in_maps, list(range(N_CORES)))
    return combine_results(res.results)


# revision 12
# speedup vs baseline: 1.4165x; 1.4165x over previous
"""Trainium2 Bass kernel for a 2-stage 13-organ Dice loss.

Math (all organ weights are 1.0, so the per-organ fold collapses to sums):
  for stage s, batch b:
    num[s,b] = 2 * sum_{c in 1..13} sum_v pred_s[b,c,v] * [target[b,v]==c]
    den[s,b] = sum_{c in 1..13} sum_v pred_s[b,c,v]^2 + count(target[b]!=0) + 13*EPS
  dice[b] = num[1,b]/den[1,b] + num[2,b]/den[2,b]
  loss    = mean_b(2 - dice[b])

Sharding: the 48-slice depth axis is split 6-per-core across 8 NeuronCores;
each core handles both batches, both stages, and organ channels 1..13
(channel 0 is background and never touches the device). Each core emits
per-partition partial sums (a few KB); the host does the final reduction
and dice division.

The kernel streams pred in bf16 (host-side cast). The loss is a ratio of
sums over ~40M elements, so the bf16 quantization noise (~1e-3 relative
per element, zero-mean) averages down to ~1e-6 on the final scalar.

Per-core device program (Tile framework; no PE/PSUM needed):
  - DVE builds the 13 one-hot masks for a whole batch's target in bf16
    with tensor_scalar(is_equal) (4x perf mode), plus a zero-count via a
    fused accum_out.
  - DVE scalar_tensor_tensor fuses (pred * 2) * mask with the
    per-partition numerator sum in one pass.
  - ACT activation(Square) computes squares with a fused per-partition
    accum_out (fp32) for the denominator.
All reductions land in small f32 "slot" tiles that are DMA'd out.
"""

import numpy as np
import ml_dtypes

import concourse.bacc as bacc
import concourse.mybir as mybir
import concourse.tile as tile
from concourse.bass_utils import run_bass_kernel_spmd

N_CORES = 8
S = 2  # stages
B = 2  # batch
C = 13  # organ channels (pred channels 1..13; channel 0 skipped)
D = 48  # depth
D_SH = D // N_CORES  # 6 depth slices per core
HW = 256 * 256  # voxels per (b, d) slab
PJ = HW // 128  # 512 free elems per partition per slab
DG = 2  # depth slices per pred tile (DMA batching)
EPS = 1e-5

F32 = mybir.dt.float32
BF16 = mybir.dt.bfloat16


def build_program(d_sh: int = D_SH, pj: int = PJ) -> bacc.Bacc:
    """Build the per-core SPMD Bass program (bf16 inputs).

    The host pre-packs inputs into the exact SBUF layout so every DMA is a
    fully contiguous block:
      pred [S, B, G, 128, C*DG*pj] bf16 — element [.., p, c*DG*pj + d*pj + j]
        = pred_orig[s, b, organ c+1, depth g*DG+d, voxel p*pj+j]
      tgt  [B, 128, d_sh*pj] bf16      — element [b, p, d*pj + j]

    Outputs (per core): onum [128,32] f32  (slot idx = (b*G + g)*S + s,
                        oden [128,32] f32   g = depth-pair group)
                        ocnt [128,16] f32  (slot idx = b*G + g; per-partition
                                            counts of target==0)
    """
    assert d_sh % DG == 0
    G = d_sh // DG
    nc = bacc.Bacc(target_bir_lowering=False)
    pred = nc.dram_tensor(
        "pred", [S, B, G, 128, C * DG * pj], BF16, kind="ExternalInput"
    )
    tgt = nc.dram_tensor("tgt", [B, 128, d_sh * pj], BF16, kind="ExternalInput")
    onum = nc.dram_tensor("onum", [128, 32], F32, kind="ExternalOutput")
    oden = nc.dram_tensor("oden", [128, 32], F32, kind="ExternalOutput")
    ocnt = nc.dram_tensor("ocnt", [128, 16], F32, kind="ExternalOutput")

    with tile.TileContext(nc) as tc:
        with (
            tc.tile_pool(name="tpool", bufs=2) as tpool,
            tc.tile_pool(name="ppool", bufs=3) as ppool,
            tc.tile_pool(name="mpool", bufs=1) as mpool,
            tc.tile_pool(name="dpool", bufs=1) as dpool,
            tc.tile_pool(name="spool", bufs=1) as spool,
        ):
            num_slots = spool.tile([128, 32], F32, tag="num")
            den_slots = spool.tile([128, 32], F32, tag="den")
            cnt_slots = spool.tile([128, 16], F32, tag="cnt")
            # Unused slot columns are DMA'd out; zero them so outputs are
            # deterministic.
            nc.vector.memset(num_slots[:, :], 0.0)
            nc.vector.memset(den_slots[:, :], 0.0)
            nc.vector.memset(cnt_slots[:, :], 0.0)

            gpj = DG * pj
            for b in range(B):
                tb = tpool.tile([128, d_sh * pj], BF16, tag="tb")
                nc.sync.dma_start(out=tb[:, :], in_=tgt[b])
                for g in range(G):
                    # 13 one-hot masks for this depth-pair's target (bf16
                    # in/out -> 4x DVE mode), matching the pred tile layout.
                    masks = mpool.tile([128, C, gpj], BF16, tag="masks")
                    for c in range(C):
                        nc.vector.tensor_scalar(
                            masks[:, c, :],
                            tb[:, g * gpj : (g + 1) * gpj],
                            float(c + 1),
                            None,
                            mybir.AluOpType.is_equal,
                        )
                    zdummy = dpool.tile([128, gpj], BF16, tag="zd")
                    nc.vector.tensor_scalar(
                        zdummy[:, :],
                        tb[:, g * gpj : (g + 1) * gpj],
                        0.0,
                        None,
                        mybir.AluOpType.is_equal,
                        mybir.AluOpType.add,
                        accum_out=cnt_slots[:, b * G + g : b * G + g + 1],
                    )
                    for s in range(S):
                        slot = (b * G + g) * S + s
                        pt = ppool.tile([128, C, gpj], BF16, tag="pt")
                        nc.sync.dma_start(out=pt[:, :, :], in_=pred[s, b, g])
                        sdummy = dpool.tile([128, C, gpj], BF16, tag="sd")
                        nc.scalar.activation(
                            sdummy[:, :, :],
                            pt[:, :, :],
                            mybir.ActivationFunctionType.Square,
                            accum_out=den_slots[:, slot : slot + 1],
                        )
                        mdummy = dpool.tile([128, C, gpj], BF16, tag="md")
                        # out = (pt * 2.0) * mask; accum_out = per-partition sum
                        nc.vector.scalar_tensor_tensor(
                            out=mdummy[:, :, :],
                            in0=pt[:, :, :],
                            scalar=2.0,
                            in1=masks[:, :, :],
                            op0=mybir.AluOpType.mult,
                            op1=mybir.AluOpType.mult,
                            accum_out=num_slots[:, slot : slot + 1],
                        )

            nc.sync.dma_start(out=onum[:, :], in_=num_slots[:, :])
            nc.sync.dma_start(out=oden[:, :], in_=den_slots[:, :])
            nc.sync.dma_start(out=ocnt[:, :], in_=cnt_slots[:, :])
    nc.finalize()
    return nc


def shard_inputs(pred_stage1, pred_stage2, target, n_cores=N_CORES, d_sh=D_SH):
    """Slice off the background channel, split depth per core, cast to bf16,
    and pack into the device layout (see build_program docstring)."""
    G = d_sh // DG
    in_maps = []
    p1 = np.asarray(pred_stage1)
    p2 = np.asarray(pred_stage2)
    tg = np.asarray(target)
    for k in range(n_cores):
        d0, d1 = k * d_sh, (k + 1) * d_sh
        pshard = np.empty((S, B, G, 128, C * DG * PJ), ml_dtypes.bfloat16)
        for s, src in enumerate((p1, p2)):
            x = src[:, 1:, d0:d1].reshape(B, C, G, DG, 128, PJ)
            x = x.transpose(0, 2, 4, 1, 3, 5)  # (B, G, 128, C, DG, PJ)
            pshard[s] = x.reshape(B, G, 128, C * DG * PJ)
        t = tg[:, d0:d1].reshape(B, d_sh, 128, PJ).transpose(0, 2, 1, 3)
        tshard = t.reshape(B, 128, d_sh * PJ).astype(ml_dtypes.bfloat16)
        in_maps.append({"pred": pshard, "tgt": tshard})
    return in_maps


def combine_results(results, d_sh=D_SH, pj=PJ):
    """Host-side final reduction of the per-core per-partition partials."""
    G = d_sh // DG
    num = np.zeros((S, B), np.float64)
    den = np.zeros((S, B), np.float64)
    cnt = np.zeros((B,), np.float64)
    group_voxels = 128 * pj * DG
    for r in results:
        onum = r["onum"].astype(np.float64)
        oden = r["oden"].astype(np.float64)
        ocnt = r["ocnt"].astype(np.float64)
        for b in range(B):
            for g in range(G):
                cnt[b] += group_voxels - ocnt[:, b * G + g].sum()
                for s in range(S):
                    slot = (b * G + g) * S + s
                    num[s, b] += onum[:, slot].sum()
                    den[s, b] += oden[:, slot].sum()
    dice = np.zeros(B, np.float64)
    for b in range(B):
        for s in range(S):
            dice[b] += num[s, b] / (den[s, b] + cnt[b] + C * EPS)
    loss = np.mean(2.0 - dice)
    return np.array(loss, dtype=np.float32)


def kernel(pred_stage1, pred_stage2, target):
    in_maps = shard_inputs(pred_stage1, pred_stage2, target)
    nc = build_program()
    res = run_bass_kernel_spmd(nc, in_maps, list(range(N_CORES)))
    return combine_results(res.results)


# revision 21
# speedup vs baseline: 1.8431x; 1.3012x over previous
"""Trainium2 Bass kernel for a 2-stage 13-organ Dice loss.

Math (all organ weights are 1.0, so the per-organ fold collapses to sums):
  for stage s, batch b:
    num[s,b] = 2 * sum_{c in 1..13} sum_v pred_s[b,c,v] * [target[b,v]==c]
    den[s,b] = sum_{c in 1..13} sum_v pred_s[b,c,v]^2 + count(target[b]!=0) + 13*EPS
  dice[b] = num[1,b]/den[1,b] + num[2,b]/den[2,b]
  loss    = mean_b(2 - dice[b])

Sharding: the 48-slice depth axis is split 6-per-core across 8 NeuronCores;
each core handles both batches, both stages, and organ channels 1..13
(channel 0 is background and never touches the device). Each core emits
per-partition partial sums (a few KB); the host does the final reduction
and dice division.

The kernel streams pred in bf16 (host-side cast). The loss is a ratio of
sums over ~40M elements, so the bf16 quantization noise (~1e-3 relative
per element, zero-mean) averages down to ~1e-6 on the final scalar.

Per-core device program (Tile framework; no PE/PSUM needed):
  - DVE builds the 13 one-hot masks for a whole batch's target in bf16
    with tensor_scalar(is_equal) (4x perf mode), plus a zero-count via a
    fused accum_out.
  - DVE scalar_tensor_tensor fuses (pred * 2) * mask with the
    per-partition numerator sum in one pass.
  - ACT activation(Square) computes squares with a fused per-partition
    accum_out (fp32) for the denominator.
All reductions land in small f32 "slot" tiles that are DMA'd out.
"""

import numpy as np
import ml_dtypes

import concourse.bacc as bacc
import concourse.mybir as mybir
import concourse.tile as tile
from concourse.bass_utils import run_bass_kernel_spmd

N_CORES = 8
S = 2  # stages
B = 2  # batch
C = 13  # organ channels (pred channels 1..13; channel 0 skipped)
D = 48  # depth
D_SH = D // N_CORES  # 6 depth slices per core
HW = 256 * 256  # voxels per (b, d) slab
PJ = HW // 128  # 512 free elems per partition per slab
DG = 2  # depth slices per pred tile (DMA batching)
EPS = 1e-5

F32 = mybir.dt.float32
BF16 = mybir.dt.bfloat16


def build_program(d_sh: int = D_SH, pj: int = PJ) -> bacc.Bacc:
    """Build the per-core SPMD Bass program (bf16 inputs).

    The host pre-packs inputs into the exact SBUF layout so every DMA is a
    fully contiguous block:
      pred [S, B, G, 128, C*DG*pj] bf16 — element [.., p, c*DG*pj + d*pj + j]
        = pred_orig[s, b, organ c+1, depth g*DG+d, voxel p*pj+j]
      tgt  [B, 128, d_sh*pj] bf16      — element [b, p, d*pj + j]

    Outputs (per core):
      onum [128, 128*S*B] f32 — per-(s,b) PSUM blocks of the TensorE
        "diagonal trick": block q = s*B + b holds M[i,j] = sum_chunks
        sum_p pred_chunk[p,i]*mask_chunk[p,j]; its DIAGONAL sums to
        sum(pred*onehot) for that (s,b). Host extracts the trace.
      oden [128,32] f32 (slot idx = (b*G + g)*S + s; per-partition sum
        of squares from the ACT accumulator)
      ocnt [128,16] f32 (slot idx = b*G + g; per-partition counts of
        target==0)
    """
    assert d_sh % DG == 0
    w = min(128, DG * pj)  # matmul chunk width (128 at full size)
    assert (DG * pj) % w == 0
    G = d_sh // DG
    K_CHUNKS = (DG * pj) // w
    nc = bacc.Bacc(target_bir_lowering=False)
    pred = nc.dram_tensor(
        "pred", [S, B, G, 128, C * DG * pj], BF16, kind="ExternalInput"
    )
    tgt = nc.dram_tensor("tgt", [B, 128, d_sh * pj], BF16, kind="ExternalInput")
    onum = nc.dram_tensor("onum", [128, 128 * S * B], F32, kind="ExternalOutput")
    oden = nc.dram_tensor("oden", [128, 32], F32, kind="ExternalOutput")
    ocnt = nc.dram_tensor("ocnt", [128, 16], F32, kind="ExternalOutput")
    # number of matmuls accumulated into each (s,b) PSUM block
    mm_total = G * C * K_CHUNKS

    with tile.TileContext(nc) as tc:
        with (
            tc.tile_pool(name="tpool", bufs=2) as tpool,
            tc.tile_pool(name="ppool", bufs=4) as ppool,
            tc.tile_pool(name="mpool", bufs=2) as mpool,
            tc.tile_pool(name="dpool", bufs=1) as dpool,
            tc.tile_pool(name="spool", bufs=1) as spool,
            tc.tile_pool(name="qpool", bufs=1, space="PSUM") as qpool,
        ):
            den_slots = spool.tile([128, 32], F32, tag="den")
            cnt_slots = spool.tile([128, 16], F32, tag="cnt")
            numsb = spool.tile([128, 128 * S * B], F32, tag="numsb")
            # Unused slot columns are DMA'd out; zero them so outputs are
            # deterministic.
            nc.vector.memset(den_slots[:, :], 0.0)
            nc.vector.memset(cnt_slots[:, :], 0.0)
            nc.vector.memset(numsb[:, :], 0.0)
            psums = {
                (s, b): qpool.tile(
                    [128, 128], F32, tag=f"ps{s}{b}", name=f"psum_{s}_{b}"
                )
                for s in range(S)
                for b in range(B)
            }
            mm_count = {k: 0 for k in psums}

            gpj = DG * pj
            for b in range(B):
                tb = tpool.tile([128, d_sh * pj], BF16, tag="tb")
                nc.sync.dma_start(out=tb[:, :], in_=tgt[b])
                for g in range(G):
                    # 13 one-hot masks for this depth-pair's target (bf16
                    # in/out -> 4x DVE mode), matching the pred tile layout.
                    masks = mpool.tile([128, C, gpj], BF16, tag="masks")
                    for c in range(C):
                        nc.vector.tensor_scalar(
                            masks[:, c, :],
                            tb[:, g * gpj : (g + 1) * gpj],
                            float(c + 1),
                            None,
                            mybir.AluOpType.is_equal,
                        )
                    zdummy = dpool.tile([128, gpj], BF16, tag="zd")
                    nc.vector.tensor_scalar(
                        zdummy[:, :],
                        tb[:, g * gpj : (g + 1) * gpj],
                        0.0,
                        None,
                        mybir.AluOpType.is_equal,
                        mybir.AluOpType.add,
                        accum_out=cnt_slots[:, b * G + g : b * G + g + 1],
                    )
                    for s in range(S):
                        slot = (b * G + g) * S + s
                        pt = ppool.tile([128, C, gpj], BF16, tag="pt")
                        nc.sync.dma_start(out=pt[:, :, :], in_=pred[s, b, g])
                        sdummy = dpool.tile([128, C, gpj], BF16, tag="sd")
                        nc.scalar.activation(
                            sdummy[:, :, :],
                            pt[:, :, :],
                            mybir.ActivationFunctionType.Square,
                            accum_out=den_slots[:, slot : slot + 1],
                        )
                        # Numerator on TensorE: accumulate
                        # sum_p pt[p,i]*mask[p,j] into the (s,b) PSUM block;
                        # only the diagonal is meaningful (host extracts it).
                        ps = psums[(s, b)]
                        for c in range(C):
                            for k in range(K_CHUNKS):
                                col = slice(k * w, (k + 1) * w)
                                mm_count[(s, b)] += 1
                                nc.tensor.matmul(
                                    ps[:w, :w],
                                    pt[:, c, col],
                                    masks[:, c, col],
                                    start=(mm_count[(s, b)] == 1),
                                    stop=(mm_count[(s, b)] == mm_total),
                                )

            for s in range(S):
                for b in range(B):
                    q = s * B + b
                    nc.vector.tensor_copy(
                        numsb[:w, q * 128 : q * 128 + w], psums[(s, b)][:w, :w]
                    )
            nc.sync.dma_start(out=onum[:, :], in_=numsb[:, :])
            nc.sync.dma_start(out=oden[:, :], in_=den_slots[:, :])
            nc.sync.dma_start(out=ocnt[:, :], in_=cnt_slots[:, :])
    nc.finalize()
    return nc


def shard_inputs(pred_stage1, pred_stage2, target, n_cores=N_CORES, d_sh=D_SH):
    """Slice off the background channel, split depth per core, cast to bf16,
    and pack into the device layout (see build_program docstring)."""
    G = d_sh // DG
    in_maps = []
    p1 = np.asarray(pred_stage1)
    p2 = np.asarray(pred_stage2)
    tg = np.asarray(target)
    for k in range(n_cores):
        d0, d1 = k * d_sh, (k + 1) * d_sh
        pshard = np.empty((S, B, G, 128, C * DG * PJ), ml_dtypes.bfloat16)
        for s, src in enumerate((p1, p2)):
            x = src[:, 1:, d0:d1].reshape(B, C, G, DG, 128, PJ)
            x = x.transpose(0, 2, 4, 1, 3, 5)  # (B, G, 128, C, DG, PJ)
            pshard[s] = x.reshape(B, G, 128, C * DG * PJ)
        t = tg[:, d0:d1].reshape(B, d_sh, 128, PJ).transpose(0, 2, 1, 3)
        tshard = t.reshape(B, 128, d_sh * PJ).astype(ml_dtypes.bfloat16)
        in_maps.append({"pred": pshard, "tgt": tshard})
    return in_maps


def combine_results(results, d_sh=D_SH, pj=PJ):
    """Host-side final reduction of the per-core per-partition partials."""
    G = d_sh // DG
    num = np.zeros((S, B), np.float64)
    den = np.zeros((S, B), np.float64)
    cnt = np.zeros((B,), np.float64)
    group_voxels = 128 * pj * DG
    for r in results:
        onum = r["onum"].astype(np.float64)
        oden = r["oden"].astype(np.float64)
        ocnt = r["ocnt"].astype(np.float64)
        for b in range(B):
            for s in range(S):
                q = s * B + b
                num[s, b] += 2.0 * np.trace(onum[:, q * 128 : (q + 1) * 128])
            for g in range(G):
                cnt[b] += group_voxels - ocnt[:, b * G + g].sum()
                for s in range(S):
                    slot = (b * G + g) * S + s
                    den[s, b] += oden[:, slot].sum()
    dice = np.zeros(B, np.float64)
    for b in range(B):
        for s in range(S):
            dice[b] += num[s, b] / (den[s, b] + cnt[b] + C * EPS)
    loss = np.mean(2.0 - dice)
    return np.array(loss, dtype=np.float32)


def kernel(pred_stage1, pred_stage2, target):
    in_maps = shard_inputs(pred_stage1, pred_stage2, target)
    nc = build_program()
    res = run_bass_kernel_spmd(nc, in_maps, list(range(N_CORES)))
    return combine_results(res.results)


# revision 29
# speedup vs baseline: 2.0304x; 1.1016x over previous
"""Trainium2 Bass kernel for a 2-stage 13-organ Dice loss.

Math (all organ weights are 1.0, so the per-organ fold collapses to sums):
  for stage s, batch b:
    num[s,b] = 2 * sum_{c in 1..13} sum_v pred_s[b,c,v] * [target[b,v]==c]
    den[s,b] = sum_{c in 1..13} sum_v pred_s[b,c,v]^2 + count(target[b]!=0) + 13*EPS
  dice[b] = num[1,b]/den[1,b] + num[2,b]/den[2,b]
  loss    = mean_b(2 - dice[b])

Sharding: the 48-slice depth axis is split 6-per-core across 8 NeuronCores;
each core handles both batches, both stages, and organ channels 1..13
(channel 0 is background and never touches the device). Each core emits
per-partition partial sums (a few KB); the host does the final reduction
and dice division.

The kernel streams pred in bf16 (host-side cast). The loss is a ratio of
sums over ~40M elements, so the bf16 quantization noise (~1e-3 relative
per element, zero-mean) averages down to ~1e-6 on the final scalar.

Per-core device program (Tile framework; no PE/PSUM needed):
  - DVE builds the 13 one-hot masks for a whole batch's target in bf16
    with tensor_scalar(is_equal) (4x perf mode), plus a zero-count via a
    fused accum_out.
  - DVE scalar_tensor_tensor fuses (pred * 2) * mask with the
    per-partition numerator sum in one pass.
  - ACT activation(Square) computes squares with a fused per-partition
    accum_out (fp32) for the denominator.
All reductions land in small f32 "slot" tiles that are DMA'd out.
"""

import numpy as np
import ml_dtypes

import concourse.bacc as bacc
import concourse.mybir as mybir
import concourse.tile as tile
from concourse.bass_utils import run_bass_kernel_spmd

N_CORES = 8
S = 2  # stages
B = 2  # batch
C = 13  # organ channels (pred channels 1..13; channel 0 skipped)
D = 48  # depth
D_SH = D // N_CORES  # 6 depth slices per core
HW = 256 * 256  # voxels per (b, d) slab
PJ = HW // 128  # 512 free elems per partition per slab
DG = 2  # depth slices per pred tile (DMA batching)
# Work split across engines (channels out of C=13). The numerator runs on
# TensorE (diag trick) for the first NPE_NUM channels and on VectorE
# (scalar_tensor_tensor) for the rest; the denominator squares run on
# ScalarE for the first NACT_DEN channels and on VectorE for the rest.
# Chosen so PE / ACT / DVE / DMA all land near the same busy time.
NPE_NUM = 11
NACT_DEN = 10
EPS = 1e-5

F32 = mybir.dt.float32
BF16 = mybir.dt.bfloat16


def build_program(d_sh: int = D_SH, pj: int = PJ) -> bacc.Bacc:
    """Build the per-core SPMD Bass program (bf16 inputs).

    The host pre-packs inputs into the exact SBUF layout so every DMA is a
    fully contiguous block:
      pred [S, B, G, 128, C*DG*pj] bf16 — element [.., p, c*DG*pj + d*pj + j]
        = pred_orig[s, b, organ c+1, depth g*DG+d, voxel p*pj+j]
      tgt  [B, 128, d_sh*pj] bf16      — element [b, p, d*pj + j]

    Outputs (per core):
      onum [128, 128*S*B] f32 — per-(s,b) PSUM blocks of the TensorE
        "diagonal trick": block q = s*B + b holds M[i,j] = sum_chunks
        sum_p pred_chunk[p,i]*mask_chunk[p,j]; its DIAGONAL sums to
        sum(pred*onehot) for that (s,b). Host extracts the trace.
      oden [128,32] f32 (slot idx = (b*G + g)*S + s; per-partition sum
        of squares of channels [0, NACT_DEN) from the ACT accumulator)
      osl  [128,64] f32 (DVE slots: col slot = numerator of channels
        [NPE_NUM, C) (incl. the 2x factor); col 32+slot = sum of squares
        of channels [NACT_DEN, C))
      ocnt [128,16] f32 (slot idx = b*G + g; per-partition counts of
        target==0)
    """
    assert d_sh % DG == 0
    w = min(128, DG * pj)  # matmul chunk width (128 at full size)
    assert (DG * pj) % w == 0
    G = d_sh // DG
    K_CHUNKS = (DG * pj) // w
    nc = bacc.Bacc(target_bir_lowering=False)
    pred = nc.dram_tensor(
        "pred", [S, B, G, 128, C * DG * pj], BF16, kind="ExternalInput"
    )
    tgt = nc.dram_tensor("tgt", [B, 128, d_sh * pj], BF16, kind="ExternalInput")
    onum = nc.dram_tensor("onum", [128, 128 * S * B], F32, kind="ExternalOutput")
    oden = nc.dram_tensor("oden", [128, 32], F32, kind="ExternalOutput")
    osl = nc.dram_tensor("osl", [128, 64], F32, kind="ExternalOutput")
    ocnt = nc.dram_tensor("ocnt", [128, 16], F32, kind="ExternalOutput")
    # number of matmuls accumulated into each (s,b) PSUM block
    mm_total = G * NPE_NUM * K_CHUNKS

    with tile.TileContext(nc) as tc:
        with (
            tc.tile_pool(name="tpool", bufs=2) as tpool,
            tc.tile_pool(name="ppool", bufs=4) as ppool,
            tc.tile_pool(name="mpool", bufs=2) as mpool,
            tc.tile_pool(name="dpool", bufs=1) as dpool,
            tc.tile_pool(name="spool", bufs=1) as spool,
            tc.tile_pool(name="qpool", bufs=1, space="PSUM") as qpool,
        ):
            den_slots = spool.tile([128, 32], F32, tag="den")
            sl_slots = spool.tile([128, 64], F32, tag="sl")
            cnt_slots = spool.tile([128, 16], F32, tag="cnt")
            numsb = spool.tile([128, 128 * S * B], F32, tag="numsb")
            # Unused slot columns are DMA'd out; zero them so outputs are
            # deterministic.
            nc.vector.memset(den_slots[:, :], 0.0)
            nc.vector.memset(sl_slots[:, :], 0.0)
            nc.vector.memset(cnt_slots[:, :], 0.0)
            nc.vector.memset(numsb[:, :], 0.0)
            psums = {
                (s, b): qpool.tile(
                    [128, 128], F32, tag=f"ps{s}{b}", name=f"psum_{s}_{b}"
                )
                for s in range(S)
                for b in range(B)
            }
            mm_count = {k: 0 for k in psums}

            gpj = DG * pj
            for b in range(B):
                tb = tpool.tile([128, d_sh * pj], BF16, tag="tb")
                nc.sync.dma_start(out=tb[:, :], in_=tgt[b])
                for g in range(G):
                    # 13 one-hot masks for this depth-pair's target (bf16
                    # in/out -> 4x DVE mode), matching the pred tile layout.
                    masks = mpool.tile([128, C, gpj], BF16, tag="masks")
                    for c in range(C):
                        nc.vector.tensor_scalar(
                            masks[:, c, :],
                            tb[:, g * gpj : (g + 1) * gpj],
                            float(c + 1),
                            None,
                            mybir.AluOpType.is_equal,
                        )
                    zdummy = dpool.tile([128, gpj], BF16, tag="zd")
                    nc.vector.tensor_scalar(
                        zdummy[:, :],
                        tb[:, g * gpj : (g + 1) * gpj],
                        0.0,
                        None,
                        mybir.AluOpType.is_equal,
                        mybir.AluOpType.add,
                        accum_out=cnt_slots[:, b * G + g : b * G + g + 1],
                    )
                    for s in range(S):
                        slot = (b * G + g) * S + s
                        pt = ppool.tile([128, C, gpj], BF16, tag="pt")
                        nc.sync.dma_start(out=pt[:, :, :], in_=pred[s, b, g])
                        # Denominator squares: ScalarE for channels
                        # [0, NACT_DEN), VectorE (STT) for the rest.
                        sdummy = dpool.tile([128, NACT_DEN, gpj], BF16, tag="sd")
                        nc.scalar.activation(
                            sdummy[:, :, :],
                            pt[:, :NACT_DEN, :],
                            mybir.ActivationFunctionType.Square,
                            accum_out=den_slots[:, slot : slot + 1],
                        )
                        sdummy2 = dpool.tile(
                            [128, C - NACT_DEN, gpj], BF16, tag="sd2"
                        )
                        nc.vector.scalar_tensor_tensor(
                            out=sdummy2[:, :, :],
                            in0=pt[:, NACT_DEN:, :],
                            scalar=1.0,
                            in1=pt[:, NACT_DEN:, :],
                            op0=mybir.AluOpType.mult,
                            op1=mybir.AluOpType.mult,
                            accum_out=sl_slots[:, 32 + slot : 32 + slot + 1],
                        )
                        # Numerator: TensorE diag trick for channels
                        # [0, NPE_NUM) — accumulate sum_p pt[p,i]*mask[p,j]
                        # into the (s,b) PSUM block (host extracts the
                        # diagonal) — VectorE STT for the rest.
                        ps = psums[(s, b)]
                        for c in range(NPE_NUM):
                            for k in range(K_CHUNKS):
                                col = slice(k * w, (k + 1) * w)
                                mm_count[(s, b)] += 1
                                nc.tensor.matmul(
                                    ps[:w, :w],
                                    pt[:, c, col],
                                    masks[:, c, col],
                                    start=(mm_count[(s, b)] == 1),
                                    stop=(mm_count[(s, b)] == mm_total),
                                )
                        mdummy = dpool.tile([128, C - NPE_NUM, gpj], BF16, tag="md")
                        nc.vector.scalar_tensor_tensor(
                            out=mdummy[:, :, :],
                            in0=pt[:, NPE_NUM:, :],
                            scalar=2.0,
                            in1=masks[:, NPE_NUM:, :],
                            op0=mybir.AluOpType.mult,
                            op1=mybir.AluOpType.mult,
                            accum_out=sl_slots[:, slot : slot + 1],
                        )

            for s in range(S):
                for b in range(B):
                    q = s * B + b
                    nc.vector.tensor_copy(
                        numsb[:w, q * 128 : q * 128 + w], psums[(s, b)][:w, :w]
                    )
            nc.sync.dma_start(out=onum[:, :], in_=numsb[:, :])
            nc.sync.dma_start(out=oden[:, :], in_=den_slots[:, :])
            nc.sync.dma_start(out=osl[:, :], in_=sl_slots[:, :])
            nc.sync.dma_start(out=ocnt[:, :], in_=cnt_slots[:, :])
    nc.finalize()
    return nc


def shard_inputs(pred_stage1, pred_stage2, target, n_cores=N_CORES, d_sh=D_SH):
    """Slice off the background channel, split depth per core, cast to bf16,
    and pack into the device layout (see build_program docstring)."""
    G = d_sh // DG
    in_maps = []
    p1 = np.asarray(pred_stage1)
    p2 = np.asarray(pred_stage2)
    tg = np.asarray(target)
    for k in range(n_cores):
        d0, d1 = k * d_sh, (k + 1) * d_sh
        pshard = np.empty((S, B, G, 128, C * DG * PJ), ml_dtypes.bfloat16)
        for s, src in enumerate((p1, p2)):
            x = src[:, 1:, d0:d1].reshape(B, C, G, DG, 128, PJ)
            x = x.transpose(0, 2, 4, 1, 3, 5)  # (B, G, 128, C, DG, PJ)
            pshard[s] = x.reshape(B, G, 128, C * DG * PJ)
        t = tg[:, d0:d1].reshape(B, d_sh, 128, PJ).transpose(0, 2, 1, 3)
        tshard = t.reshape(B, 128, d_sh * PJ).astype(ml_dtypes.bfloat16)
        in_maps.append({"pred": pshard, "tgt": tshard})
    return in_maps


def combine_results(results, d_sh=D_SH, pj=PJ):
    """Host-side final reduction of the per-core per-partition partials."""
    G = d_sh // DG
    num = np.zeros((S, B), np.float64)
    den = np.zeros((S, B), np.float64)
    cnt = np.zeros((B,), np.float64)
    group_voxels = 128 * pj * DG
    for r in results:
        onum = r["onum"].astype(np.float64)
        oden = r["oden"].astype(np.float64)
        osl = r["osl"].astype(np.float64)
        ocnt = r["ocnt"].astype(np.float64)
        for b in range(B):
            for s in range(S):
                q = s * B + b
                num[s, b] += 2.0 * np.trace(onum[:, q * 128 : (q + 1) * 128])
            for g in range(G):
                cnt[b] += group_voxels - ocnt[:, b * G + g].sum()
                for s in range(S):
                    slot = (b * G + g) * S + s
                    num[s, b] += osl[:, slot].sum()
                    den[s, b] += oden[:, slot].sum() + osl[:, 32 + slot].sum()
    dice = np.zeros(B, np.float64)
    for b in range(B):
        for s in range(S):
            dice[b] += num[s, b] / (den[s, b] + cnt[b] + C * EPS)
    loss = np.mean(2.0 - dice)
    return np.array(loss, dtype=np.float32)


def kernel(pred_stage1, pred_stage2, target):
    in_maps = shard_inputs(pred_stage1, pred_stage2, target)
    nc = build_program()
    # The first multi-core execution of a freshly loaded NEFF occasionally
    # hits a transient NRT_EXEC_UNIT_UNRECOVERABLE; a retry succeeds.
    last_err = None
    for _ in range(3):
        try:
            res = run_bass_kernel_spmd(nc, in_maps, list(range(N_CORES)))
            return combine_results(res.results)
        except Exception as e:  # noqa: BLE001
            last_err = e
    raise last_err


# revision 39
# speedup vs baseline: 2.1960x; 1.0815x over previous
"""Trainium2 Bass kernel for a 2-stage 13-organ Dice loss.

Math (all organ weights are 1.0, so the per-organ fold collapses to sums):
  for stage s, batch b:
    num[s,b] = 2 * sum_{c in 1..13} sum_v pred_s[b,c,v] * [target[b,v]==c]
    den[s,b] = sum_{c in 1..13} sum_v pred_s[b,c,v]^2 + count(target[b]!=0) + 13*EPS
  dice[b] = num[1,b]/den[1,b] + num[2,b]/den[2,b]
  loss    = mean_b(2 - dice[b])

Sharding: the 48-slice depth axis is split 6-per-core across 8 NeuronCores;
each core handles both batches, both stages, and organ channels 1..13
(channel 0 is background and never touches the device). Each core emits
per-partition partial sums (a few KB); the host does the final reduction
and dice division.

The kernel streams pred in bf16 (host-side cast). The loss is a ratio of
sums over ~40M elements, so the bf16 quantization noise (~1e-3 relative
per element, zero-mean) averages down to ~1e-6 on the final scalar.

Per-core device program (Tile framework; no PE/PSUM needed):
  - DVE builds the 13 one-hot masks for a whole batch's target in bf16
    with tensor_scalar(is_equal) (4x perf mode), plus a zero-count via a
    fused accum_out.
  - DVE scalar_tensor_tensor fuses (pred * 2) * mask with the
    per-partition numerator sum in one pass.
  - ACT activation(Square) computes squares with a fused per-partition
    accum_out (fp32) for the denominator.
All reductions land in small f32 "slot" tiles that are DMA'd out.
"""

import numpy as np
import ml_dtypes

import concourse.bacc as bacc
import concourse.mybir as mybir
import concourse.tile as tile
from concourse.bass_utils import run_bass_kernel_spmd

N_CORES = 8
S = 2  # stages
B = 2  # batch
C = 13  # organ channels (pred channels 1..13; channel 0 skipped)
D = 48  # depth
D_SH = D // N_CORES  # 6 depth slices per core
HW = 256 * 256  # voxels per (b, d) slab
PJ = HW // 128  # 512 free elems per partition per slab
DG = 2  # depth slices per pred tile (DMA batching)
# Work split across engines (channels out of C=13). The full numerator runs
# on TensorE: each one-hot mask chunk is loaded as the stationary operand
# once and multiplied against BOTH stages' pred chunks in a single N=256
# matmul (stationary reuse halves the LDWEIGHTS bill). The denominator
# squares run on ScalarE for the first NACT_DEN channels and on VectorE
# (scalar_tensor_tensor) for the rest. Chosen so PE / ACT / DVE / DMA all
# land near the same busy time.
NACT_DEN = 9
EPS = 1e-5

F32 = mybir.dt.float32
BF16 = mybir.dt.bfloat16


def build_program(d_sh: int = D_SH, pj: int = PJ) -> bacc.Bacc:
    """Build the per-core SPMD Bass program (bf16 inputs).

    The host pre-packs inputs into the exact SBUF layout so every DMA is a
    fully contiguous block:
      pred [S, B, G, 128, C*DG*pj] bf16 — element [.., p, c*DG*pj + d*pj + j]
        = pred_orig[s, b, organ c+1, depth g*DG+d, voxel p*pj+j]
      tgt  [B, 128, d_sh*pj] bf16      — element [b, p, d*pj + j]

    Outputs (per core):
      onum [128, 128*S*B] f32 — PSUM blocks of the TensorE "diagonal
        trick": cols [(b*S+s)*128, +128) hold M[i,j] = sum_chunks
        sum_p mask_chunk[p,i]*pred_chunk[p,j]; the DIAGONAL sums to
        sum(pred*onehot) for that (s,b). Host extracts the trace.
      oden [128,32] f32 (slot idx = (b*G + g)*S + s; per-partition sum
        of squares of channels [0, NACT_DEN) from the ACT accumulator)
      osl  [128,64] f32 (DVE slots: col 32+slot = sum of squares of
        channels [NACT_DEN, C); cols 0-31 are zero)
      ocnt [128,16] f32 (slot idx = b*G + g; per-partition counts of
        target==0)
    """
    assert d_sh % DG == 0
    w = min(128, DG * pj)  # matmul chunk width (128 at full size)
    assert (DG * pj) % w == 0
    G = d_sh // DG
    K_CHUNKS = (DG * pj) // w
    nc = bacc.Bacc(target_bir_lowering=False)
    pred = nc.dram_tensor(
        "pred", [S, B, G, 128, C * DG * pj], BF16, kind="ExternalInput"
    )
    tgt = nc.dram_tensor("tgt", [B, 128, d_sh * pj], BF16, kind="ExternalInput")
    onum = nc.dram_tensor("onum", [128, 128 * S * B], F32, kind="ExternalOutput")
    oden = nc.dram_tensor("oden", [128, 32], F32, kind="ExternalOutput")
    osl = nc.dram_tensor("osl", [128, 64], F32, kind="ExternalOutput")
    ocnt = nc.dram_tensor("ocnt", [128, 16], F32, kind="ExternalOutput")
    # number of matmuls accumulated into each per-b PSUM block
    mm_total = G * C * K_CHUNKS

    with tile.TileContext(nc) as tc:
        with (
            tc.tile_pool(name="tpool", bufs=2) as tpool,
            tc.tile_pool(name="ppool", bufs=2) as ppool,
            tc.tile_pool(name="mpool", bufs=2) as mpool,
            tc.tile_pool(name="dpool", bufs=1) as dpool,
            tc.tile_pool(name="spool", bufs=1) as spool,
            tc.tile_pool(name="qpool", bufs=1, space="PSUM") as qpool,
        ):
            den_slots = spool.tile([128, 32], F32, tag="den")
            sl_slots = spool.tile([128, 64], F32, tag="sl")
            cnt_slots = spool.tile([128, 16], F32, tag="cnt")
            numsb = spool.tile([128, 128 * S * B], F32, tag="numsb")
            # Unused slot columns are DMA'd out; zero them so outputs are
            # deterministic.
            nc.vector.memset(den_slots[:, :], 0.0)
            nc.vector.memset(sl_slots[:, :], 0.0)
            nc.vector.memset(cnt_slots[:, :], 0.0)
            nc.vector.memset(numsb[:, :], 0.0)
            psums = {
                b: qpool.tile([128, S * 128], F32, tag=f"ps{b}", name=f"psum_{b}")
                for b in range(B)
            }
            mm_count = {k: 0 for k in psums}

            gpj = DG * pj
            for b in range(B):
                tb = tpool.tile([128, d_sh * pj], BF16, tag="tb")
                nc.sync.dma_start(out=tb[:, :], in_=tgt[b])
                for g in range(G):
                    # 13 one-hot masks for this depth-pair's target (bf16
                    # in/out -> 4x DVE mode), matching the pred tile layout.
                    masks = mpool.tile([128, C, gpj], BF16, tag="masks")
                    for c in range(C):
                        nc.vector.tensor_scalar(
                            masks[:, c, :],
                            tb[:, g * gpj : (g + 1) * gpj],
                            float(c + 1),
                            None,
                            mybir.AluOpType.is_equal,
                        )
                    zdummy = dpool.tile([128, gpj], BF16, tag="zd")
                    nc.vector.tensor_scalar(
                        zdummy[:, :],
                        tb[:, g * gpj : (g + 1) * gpj],
                        0.0,
                        None,
                        mybir.AluOpType.is_equal,
                        mybir.AluOpType.add,
                        accum_out=cnt_slots[:, b * G + g : b * G + g + 1],
                    )
                    # One DMA brings BOTH stages' (b,g) pred block.
                    pt = ppool.tile([128, S, C * gpj], BF16, tag="pt")
                    nc.sync.dma_start(
                        out=pt[:, :, :],
                        in_=pred[:, b, g].rearrange("s p f -> p s f"),
                    )
                    for s in range(S):
                        slot = (b * G + g) * S + s
                        # Denominator squares: ScalarE for channels
                        # [0, NACT_DEN), VectorE (STT) for the rest.
                        sdummy = dpool.tile([128, NACT_DEN * gpj], BF16, tag="sd")
                        nc.scalar.activation(
                            sdummy[:, :],
                            pt[:, s, : NACT_DEN * gpj],
                            mybir.ActivationFunctionType.Square,
                            accum_out=den_slots[:, slot : slot + 1],
                        )
                        sdummy2 = dpool.tile(
                            [128, (C - NACT_DEN) * gpj], BF16, tag="sd2"
                        )
                        nc.vector.scalar_tensor_tensor(
                            out=sdummy2[:, :],
                            in0=pt[:, s, NACT_DEN * gpj :],
                            scalar=1.0,
                            in1=pt[:, s, NACT_DEN * gpj :],
                            op0=mybir.AluOpType.mult,
                            op1=mybir.AluOpType.mult,
                            accum_out=sl_slots[:, 32 + slot : 32 + slot + 1],
                        )
                    # Numerator on TensorE: load each mask chunk as the
                    # stationary ONCE and stream both stages' pred chunks
                    # as one N=2*w moving operand; accumulate into the
                    # per-b PSUM block (host extracts the diagonals).
                    ps = psums[b]
                    for c in range(C):
                        for k in range(K_CHUNKS):
                            col = slice(c * gpj + k * w, c * gpj + (k + 1) * w)
                            mm_count[b] += 1
                            nc.tensor.matmul(
                                ps[:w, : S * w],
                                masks[:, c, k * w : (k + 1) * w],
                                pt[:, :, col],
                                start=(mm_count[b] == 1),
                                stop=(mm_count[b] == mm_total),
                            )

            for b in range(B):
                for s in range(S):
                    q = b * S + s
                    nc.vector.tensor_copy(
                        numsb[:w, q * 128 : q * 128 + w],
                        psums[b][:w, s * w : s * w + w],
                    )
            nc.sync.dma_start(out=onum[:, :], in_=numsb[:, :])
            nc.sync.dma_start(out=oden[:, :], in_=den_slots[:, :])
            nc.sync.dma_start(out=osl[:, :], in_=sl_slots[:, :])
            nc.sync.dma_start(out=ocnt[:, :], in_=cnt_slots[:, :])
    nc.finalize()
    return nc


def shard_inputs(pred_stage1, pred_stage2, target, n_cores=N_CORES, d_sh=D_SH):
    """Slice off the background channel, split depth per core, cast to bf16,
    and pack into the device layout (see build_program docstring)."""
    G = d_sh // DG
    in_maps = []
    p1 = np.asarray(pred_stage1)
    p2 = np.asarray(pred_stage2)
    tg = np.asarray(target)
    for k in range(n_cores):
        d0, d1 = k * d_sh, (k + 1) * d_sh
        pshard = np.empty((S, B, G, 128, C * DG * PJ), ml_dtypes.bfloat16)
        for s, src in enumerate((p1, p2)):
            x = src[:, 1:, d0:d1].reshape(B, C, G, DG, 128, PJ)
            x = x.transpose(0, 2, 4, 1, 3, 5)  # (B, G, 128, C, DG, PJ)
            pshard[s] = x.reshape(B, G, 128, C * DG * PJ)
        t = tg[:, d0:d1].reshape(B, d_sh, 128, PJ).transpose(0, 2, 1, 3)
        tshard = t.reshape(B, 128, d_sh * PJ).astype(ml_dtypes.bfloat16)
        in_maps.append({"pred": pshard, "tgt": tshard})
    return in_maps


def combine_results(results, d_sh=D_SH, pj=PJ):
    """Host-side final reduction of the per-core per-partition partials."""
    G = d_sh // DG
    num = np.zeros((S, B), np.float64)
    den = np.zeros((S, B), np.float64)
    cnt = np.zeros((B,), np.float64)
    group_voxels = 128 * pj * DG
    for r in results:
        onum = r["onum"].astype(np.float64)
        oden = r["oden"].astype(np.float64)
        osl = r["osl"].astype(np.float64)
        ocnt = r["ocnt"].astype(np.float64)
        for b in range(B):
            for s in range(S):
                q = b * S + s
                num[s, b] += 2.0 * np.trace(onum[:, q * 128 : (q + 1) * 128])
            for g in range(G):
                cnt[b] += group_voxels - ocnt[:, b * G + g].sum()
                for s in range(S):
                    slot = (b * G + g) * S + s
                    num[s, b] += osl[:, slot].sum()
                    den[s, b] += oden[:, slot].sum() + osl[:, 32 + slot].sum()
    dice = np.zeros(B, np.float64)
    for b in range(B):
        for s in range(S):
            dice[b] += num[s, b] / (den[s, b] + cnt[b] + C * EPS)
    loss = np.mean(2.0 - dice)
    return np.array(loss, dtype=np.float32)


def kernel(pred_stage1, pred_stage2, target):
    in_maps = shard_inputs(pred_stage1, pred_stage2, target)
    nc = build_program()
    # The first multi-core execution of a freshly loaded NEFF occasionally
    # hits a transient NRT_EXEC_UNIT_UNRECOVERABLE; a retry succeeds.
    last_err = None
    for _ in range(3):
        try:
            res = run_bass_kernel_spmd(nc, in_maps, list(range(N_CORES)))
            return combine_results(res.results)
        except Exception as e:  # noqa: BLE001
            last_err = e
    raise last_err
